# revision 9
# baseline (speedup 1.0000x reference)
"""Trainium2 Bass kernel: CQT (constant-Q transform) of 2^23 audio samples.

Reference math (jax):
    frames[f, n] = x[f*HOP + n]                  HOP=512, fftLen=2048
    four_r = frames @ wcos.T ; four_i = frames @ wsin.T
    cqt_r  = kr @ four_r - ki @ four_i
    cqt_i  = kr @ four_i + ki @ four_r
    out    = sqrt(cqt_r**2 + cqt_i**2)           # [1, 84, n_frames]

Folded on the host (exact algebra, tiny matrices):
    A = kr@wcos - ki@wsin,  B = kr@wsin + ki@wcos      (each [84, 2048])
    out = sqrt((A @ frames.T)**2 + (B @ frames.T)**2)

Support truncation: the folded CQT kernels are time-compact (bin j spans
~1686*2^(-j/12) samples centered at 1024) apart from a diffuse leakage
floor from the half-spectrum fold.  Bins are permuted (ORDER) so that for
every 128-sample contraction chunk kc the active bins are a prefix; chunk
kc streams only W[kc] bin-pairs.  Chunks 0 and 15 are dropped entirely.
Streamed columns per frame-chunk fall 2688 -> 1144 (measured MM pacing is
N/2.4GHz + ~5ns, LDWEIGHTS hidden by the background weight buffer).

Device strategy (8-way shard along the frame axis; kernels replicated):
  - 2048 frames per core, bf16, deinterleaved x layout: plane r of frame
    block fb holds x blocks congruent to r mod 4 as columns, so every
    matmul operand is a contiguous column range.
  - matmuls: 128-frame x-chunks stationary (full PE width, FWL), the
    interleaved [a_j b_j ...] chunk of A/B streams; 14 chunks accumulate
    into a psum prefix; psum holds TWO frame-chunks (parity pairs) per
    bank so one big ACT square + one DVE add serve 2 fc (ACT otherwise
    bottlenecks); outputs leave as bf16 |cqt|^2, sqrt on the host.
  - overlap: AB kernel block DMAs first (both rings), then fb0 planes in
    the order the chunk loop consumes them; junk matmuls on raw SBUF
    preheat the PE clock (HAM) while the first chunks land.
  - post-passes for this toolchain: multi-wait instructions are split onto
    injected NoOps (walrus encodes at most ONE sem wait per instruction),
    non-group-end matmul PE-sem increments are stripped (PE sequencer
    retires incs at ~115ns), and the Tile entry/exit all-engine barriers
    are elided (single-shot NEFF; the SP drain still waits every proc).
"""

import sys

if "/opt/trn_rl_repo" not in sys.path:
    sys.path.insert(0, "/opt/trn_rl_repo")

import numpy as np
import ml_dtypes

HOP = 512
FFTLEN = 2048
N_BINS = 84
T_SAMPLES = 8388608
N_FRAMES = (T_SAMPLES - FFTLEN) // HOP + 1  # 16381
N_CORES = 8
F_PER_CORE = 2048                 # frames computed per core (3 junk at the end)
X_COLS_TOTAL = 8204               # sample columns actually needed per core
SHARD_LEN = X_COLS_TOTAL * 128    # 1050112 samples per core
CORE_STRIDE = F_PER_CORE * HOP    # 1048576 samples between shard starts
N_FC = F_PER_CORE // 128          # 16 output frame chunks (128 frames each)
PLANE_COLS = 515                  # columns per r-plane per frame block
FB_COLS = 4 * PLANE_COLS          # 2060
N_FB = 4                          # frame blocks of 512 frames

# bins permuted so per-chunk active sets are prefixes (widest span first)
ORDER = [0, 1, 66, 67, 68, 69, 78, 79, 80, 81, 2, 15, 16, 17, 3, 4, 5, 6,
         18, 19, 20, 21, 22, 7, 8, 9, 10, 11, 23, 24, 25, 26, 27, 82, 12,
         13, 14, 28, 29, 30, 31, 32, 33, 34, 35, 36, 37, 38, 39, 40, 41,
         42, 43, 44, 45, 46, 47, 48, 49, 51, 52, 53, 54, 55, 56, 57, 58,
         59, 60, 61, 62, 63, 64, 65, 70, 83, 50, 71, 72, 73, 74, 75, 76,
         77]
W = [0, 0, 14, 23, 34, 45, 76, 84, 84, 76, 45, 34, 23, 14, 0, 0]
# chunk issue order: full-width center chunks first (the start=True matmul
# must cover all 168 psum columns), then outward by decreasing width
CHUNK_ORDER = [8, 7, 9, 6, 10, 5, 11, 4, 12, 3, 13, 2]
AB_COLS = 2 * sum(W)              # 1144
AB_OFF = {}
_off = 0
for _kc in CHUNK_ORDER:
    AB_OFF[_kc] = _off
    _off += 2 * W[_kc]
PL0 = AB_COLS                     # planes start here
PLANE_POS = {0: 0, 1: 1, 3: 2, 2: 3}   # ring A: [p0|p1], ring B: [p3|p2]
EXT_COLS = PL0 + N_FB * FB_COLS
N_JUNK = 8                        # PE-preheat matmuls before real data lands

_PROGRAM = None


def _thin_pe_incs(nc, mybir):
    """Matmuls complete in pc order, so only each accumulation group's last
    matmul needs its PE-semaphore increment.  The PE sequencer retires incs
    at ~115ns each.  Strip non-stop matmul incs and renumber every wait on
    that semaphore."""
    sem_id = None
    tick = 0
    kept = 0
    tick_to_kept = {0: 0}
    for f in nc.m.functions:
        for blk in f.blocks:
            for inst in blk.instructions:
                si = getattr(inst, "sync_info", None)
                if si is None:
                    continue
                pe_ups = [u for u in si.on_update
                          if u.ant_name.startswith("PE")]
                if not pe_ups:
                    continue
                if type(inst).__name__ != "InstMatmult":
                    return  # unexpected PE-sem producer; skip optimization
                sem_id = pe_ups[0].id
                tick += 1
                if inst.stop_tensor_calc:
                    kept += 1
                else:
                    inst.sync_info = mybir.SyncInfo(
                        on_wait=list(si.on_wait),
                        on_update=[u for u in si.on_update
                                   if not u.ant_name.startswith("PE")])
                tick_to_kept[tick] = kept
    if sem_id is None:
        return
    for f in nc.m.functions:
        for blk in f.blocks:
            for inst in blk.instructions:
                si = getattr(inst, "sync_info", None)
                if si is None:
                    continue
                changed = False
                new_waits = []
                for w in si.on_wait:
                    if w.id == sem_id and w.wait_value in tick_to_kept:
                        nv = tick_to_kept[w.wait_value]
                        if nv != w.wait_value:
                            w = mybir.SyncWait(
                                sync_type=w.sync_type, id=w.id,
                                ant_name=w.ant_name, wait_mode=w.wait_mode,
                                wait_value=nv, wait_reg=w.wait_reg)
                            changed = True
                    new_waits.append(w)
                if changed:
                    inst.sync_info = mybir.SyncInfo(
                        on_wait=new_waits, on_update=list(si.on_update))


def _split_multi_waits(nc, mybir, max_waits=1):
    """This walrus build encodes at most one sem wait per instruction; move
    extra waits onto injected same-engine NoOps right before the instruction."""
    ctr = 0
    for f in nc.m.functions:
        for blk in f.blocks:
            il = list(blk.instructions)
            new = []
            changed = False
            for inst in il:
                si = getattr(inst, "sync_info", None)
                if si is not None and len(si.on_wait) > max_waits:
                    waits = list(si.on_wait)
                    for w in waits[:-max_waits]:
                        nop = mybir.InstNoOp(name=f"I-waitfix-{ctr}", ins=[], outs=[])
                        ctr += 1
                        nop.engine = inst.engine
                        nop.sync_info = mybir.SyncInfo(on_wait=[w], on_update=[])
                        new.append(nop)
                    inst.sync_info = mybir.SyncInfo(
                        on_wait=waits[-max_waits:], on_update=list(si.on_update))
                    changed = True
                new.append(inst)
            if changed:
                blk.instructions = new


def _build_program():
    import concourse.bass as bass
    import concourse.tile as tile
    from concourse import mybir
    from concourse.vector_clock import ScopedClock

    def _lean_drain(self, tick_clock, wait_clock):
        # Tail for a single-shot NEFF: the SP drain already waits on every
        # proc's final tick (incl. output-DMA completion).  The stock
        # drain+barrier+sem-reset+barrier tail costs ~7us and only matters
        # for re-executing a loaded NEFF with dirty semaphores.
        drain_inst = self.nc.sync.drain()
        wait_clock.add_sem_waits(
            drain_inst.ins, ScopedClock({None: tick_clock.global_clock}))
        popped = self.nc._tile_sem_poison_stack.pop()
        assert popped is self._sem_poison

    tile.TileContext._drain_and_barrier = _lean_drain

    # Skip the ~3.4us entry all-engine barrier: it orders the preamble's
    # const-AP writes (PE, t~0.4us) and SWDGE scratch memsets against the
    # body.  This kernel reads const APs first at ~13us (ACT square bias)
    # and issues no SWDGE DMAs, so engine start-skew cannot race it.
    _orig_barrier = bass.Bass.all_engine_barrier
    bass.Bass.all_engine_barrier = lambda self, **kw: None
    try:
        nc = bass.Bass("TRN2", target_bir_lowering=False, debug=False)
    finally:
        bass.Bass.all_engine_barrier = _orig_barrier

    ext = nc.dram_tensor("ext", [128, EXT_COLS], mybir.dt.bfloat16,
                         kind="ExternalInput").ap()
    # out[p, fc*84+j'] = |cqt|^2 at frame fc*128+p, permuted bin j'
    out = nc.dram_tensor("out", [128, N_FC * N_BINS], mybir.dt.bfloat16,
                         kind="ExternalOutput").ap()

    with tile.TileContext(nc) as tc:
        with (
            tc.tile_pool(name="const", bufs=1) as const,
            tc.tile_pool(name="psum", bufs=4, space="PSUM") as psum,
            tc.tile_pool(name="tmp", bufs=4) as tmp,
            tc.tile_pool(name="outp", bufs=1) as outp,
        ):
            xt = const.tile([128, EXT_COLS], mybir.dt.bfloat16)
            # chunked input on both HWDGE rings (SP + ACT issue in parallel).
            # ring A (ACT, reaches its first DMA earliest): the AB block for
            # the first two chunks, the rest of AB, then planes 0/1 of fb0;
            # ring B (SP): planes 3/2 of fb0; per-fb halves after that.
            engA, engB = nc.scalar, nc.sync
            engA.dma_start(xt[:, 0:336], ext[:, 0:336])
            engB.dma_start(xt[:, 336:AB_COLS], ext[:, 336:AB_COLS])
            fb0 = PL0
            engA.dma_start(xt[:, fb0:fb0 + PLANE_COLS],
                           ext[:, fb0:fb0 + PLANE_COLS])          # p0
            engB.dma_start(xt[:, fb0 + 2 * PLANE_COLS:fb0 + 3 * PLANE_COLS],
                           ext[:, fb0 + 2 * PLANE_COLS:fb0 + 3 * PLANE_COLS])  # p3
            engA.dma_start(xt[:, fb0 + PLANE_COLS:fb0 + 2 * PLANE_COLS],
                           ext[:, fb0 + PLANE_COLS:fb0 + 2 * PLANE_COLS])      # p1
            engB.dma_start(xt[:, fb0 + 3 * PLANE_COLS:fb0 + FB_COLS],
                           ext[:, fb0 + 3 * PLANE_COLS:fb0 + FB_COLS])         # p2
            half = FB_COLS // 2
            for fb in range(1, N_FB):
                lo = PL0 + fb * FB_COLS
                engA.dma_start(xt[:, lo:lo + half], ext[:, lo:lo + half])
                engB.dma_start(xt[:, lo + half:lo + FB_COLS],
                               ext[:, lo + half:lo + FB_COLS])

            # PE preheat: junk matmuls on raw (uninitialized, untracked) SBUF
            # keep the PE busy from the first post-preamble cycle, so HAM is
            # at full clock when the real matmuls start
            junk = nc.alloc_sbuf_tensor("junk", [128, 512],
                                        mybir.dt.bfloat16).ap()
            for _ in range(N_JUNK):
                ps_w = psum.tile([128, 512], mybir.dt.float32, tag="ps")
                nc.tensor.matmul(ps_w[:], junk[:, :128], junk[:],
                                 start=True, stop=True, skip_group_check=True)

            o = outp.tile([128, N_FC, N_BINS], mybir.dt.bfloat16)

            def mm(ps, par, fc, kc, start, stop):
                a_, r_ = divmod(kc, 4)
                w2 = 2 * W[kc]
                fb, fi = divmod(fc, 4)  # frame block, 128-frame chunk within
                lo = (PL0 + fb * FB_COLS + PLANE_POS[r_] * PLANE_COLS
                      + fi * 128 + a_)
                lhs = xt[:, lo:lo + 128]              # x frames as weights
                rhs = xt[:, AB_OFF[kc]:AB_OFF[kc] + w2]
                nc.tensor.matmul(ps[:, par, :w2], lhs, rhs,
                                 start=start, stop=stop)

            def magnitude_pair(ps, fcpair):
                # |cqt|^2 for two frame chunks: one big square on ScalarE,
                # the interleaved-pair add on DVE (bf16: 2x rate)
                sq = tmp.tile([128, 2, N_BINS, 2], mybir.dt.bfloat16,
                              tag="sq")
                nc.scalar.square(
                    sq.rearrange("p s b t -> p (s b t)"),
                    ps.rearrange("p s c -> p (s c)"))
                nc.vector.tensor_add(o[:, 2 * fcpair:2 * fcpair + 2, :],
                                     sq[:, :, :, 0], sq[:, :, :, 1])

            def magnitude_one(ps, par, fc):
                # per-frame-chunk variant: shortens the post-last-matmul
                # critical chain for the final pair
                sq = tmp.tile([128, N_BINS, 2], mybir.dt.bfloat16, tag="sq1")
                nc.scalar.square(sq.rearrange("p b t -> p (b t)"),
                                 ps[:, par, :])
                nc.vector.tensor_add(o[:, fc, :], sq[:, :, 0], sq[:, :, 1])

            def out_dma(glo, ghi):
                # |cqt|^2 rows for fcs [glo, ghi) leave as bf16
                nc.sync.dma_start(
                    out[:, glo * N_BINS:ghi * N_BINS],
                    o[:, glo:ghi, :].rearrange("p a b -> p (a b)"))

            nck = len(CHUNK_ORDER)
            for fcpair in range(N_FC // 2):
                last = fcpair == N_FC // 2 - 1
                ps = psum.tile([128, 2, 2 * N_BINS], mybir.dt.float32,
                               tag="ps")
                for par in range(2):
                    for ci, kc in enumerate(CHUNK_ORDER):
                        mm(ps, par, 2 * fcpair + par, kc,
                           start=(ci == 0), stop=(ci == nck - 1))
                    if last:
                        magnitude_one(ps, par, 2 * fcpair + par)
                if not last:
                    magnitude_pair(ps, fcpair)
                if fcpair == 1:
                    out_dma(0, 4)
                elif fcpair == 3:
                    out_dma(4, 8)
                elif fcpair == 5:
                    out_dma(8, 12)
                elif fcpair == 6:
                    out_dma(12, 14)
                elif fcpair == 7:
                    out_dma(14, 16)

    _thin_pe_incs(nc, mybir)
    _split_multi_waits(nc, mybir)
    return nc


def _get_program():
    global _PROGRAM
    if _PROGRAM is None:
        _PROGRAM = _build_program()
    return _PROGRAM


def _host_prep(x, wcos, wsin, kr, ki):
    """Fold the CQT kernels; shard, cast, and lay out the waveform."""
    kr64 = np.asarray(kr, dtype=np.float64)
    ki64 = np.asarray(ki, dtype=np.float64)
    wc64 = np.asarray(wcos, dtype=np.float64)
    ws64 = np.asarray(wsin, dtype=np.float64)
    a = kr64 @ wc64 - ki64 @ ws64            # [84, 2048]
    b = kr64 @ ws64 + ki64 @ wc64            # [84, 2048]
    ordr = np.asarray(ORDER)
    a2 = a[ordr]
    b2 = b[ordr]
    # AB block: per chunk kc (in CHUNK_ORDER) the first W[kc] bins as
    # interleaved pairs [a_j b_j a_j b_j ...], bf16
    abblk = np.empty((128, AB_COLS), dtype=ml_dtypes.bfloat16)
    for kc in CHUNK_ORDER:
        w = W[kc]
        sl = slice(128 * kc, 128 * (kc + 1))
        pair = np.empty((128, 2 * w), dtype=np.float64)
        pair[:, 0::2] = a2[:w, sl].T
        pair[:, 1::2] = b2[:w, sl].T
        abblk[:, AB_OFF[kc]:AB_OFF[kc] + 2 * w] = pair.astype(
            ml_dtypes.bfloat16)

    x = np.asarray(x, dtype=np.float32)
    x_pad = np.zeros((N_CORES - 1) * CORE_STRIDE + SHARD_LEN, dtype=np.float32)
    x_pad[:T_SAMPLES] = x
    x_bf = x_pad.astype(ml_dtypes.bfloat16)
    exts = []
    for c in range(N_CORES):
        shard = x_bf[c * CORE_STRIDE: c * CORE_STRIDE + SHARD_LEN]
        # zz[j, r, p] = x[(4j+r)*128 + p]
        zz = shard.reshape(X_COLS_TOTAL // 4, 4, 128)
        ext = np.empty((128, EXT_COLS), dtype=ml_dtypes.bfloat16)
        ext[:, :AB_COLS] = abblk
        for fb in range(N_FB):
            for r in range(4):
                lo = PL0 + fb * FB_COLS + PLANE_POS[r] * PLANE_COLS
                ext[:, lo:lo + PLANE_COLS] = (
                    zz[fb * 512: fb * 512 + PLANE_COLS, r, :].T)
        exts.append(ext)
    return exts


_LAST_RESULTS = None  # BassKernelResults of the most recent run (for profiling)


def _ensure_ntff_hook():
    """The image's antenv lacks axon_hooks; recreate it from trn_agent_boot so
    a BASS_TRACE env (set by us or a harness) can't crash the import inside
    run_bass_kernel_spmd."""
    import types

    try:
        import antenv.axon_hooks  # noqa: F401
        return
    except ImportError:
        pass
    try:
        if "/root/.axon_site" not in sys.path:
            sys.path.insert(0, "/root/.axon_site")
        from trn_agent_boot.trn_boot import _ntff_profile_via_ctypes

        hook = _ntff_profile_via_ctypes("/opt/axon/libaxon_pjrt.so")
    except Exception:
        hook = None
    try:
        import antenv

        mod = types.ModuleType("antenv.axon_hooks")
        mod._hook = hook
        mod.get_axon_ntff_profile_hook = lambda: mod._hook
        mod.set_axon_ntff_profile_hook = lambda h: setattr(mod, "_hook", h)
        antenv.axon_hooks = mod
        sys.modules["antenv.axon_hooks"] = mod
    except Exception:
        pass


def kernel(x, wcos, wsin, kr, ki):
    global _LAST_RESULTS
    _ensure_ntff_hook()
    from concourse.bass_utils import run_bass_kernel_spmd

    exts = _host_prep(x, wcos, wsin, kr, ki)
    nc = _get_program()
    in_maps = [{"ext": exts[c]} for c in range(N_CORES)]
    res = run_bass_kernel_spmd(nc, in_maps, core_ids=list(range(N_CORES)))
    _LAST_RESULTS = res
    # per core: out[p, fc*84+j'] -> [84, 2048 frames] with frame = fc*128+p
    parts = []
    for c in range(N_CORES):
        oc = np.asarray(res.results[c]["out"], dtype=np.float32)
        oc = oc.reshape(128, N_FC, N_BINS)
        parts.append(oc.transpose(2, 1, 0).reshape(N_BINS, F_PER_CORE))
    full = np.concatenate(parts, axis=1)
    unscr = np.empty_like(full)
    unscr[np.asarray(ORDER)] = full          # row j' of full is bin ORDER[j']
    return np.sqrt(unscr[None, :, :N_FRAMES]).astype(np.float32)
